# revision 1
# baseline (speedup 1.0000x reference)
"""Trainium2 Bass kernel for a Keras SimpleRNN wrapper:

    xproj = inputs @ Wx + b                    # [B, T, H]
    h_t   = tanh(xproj_t + h_{t-1} @ Wh)       # scan over T, h_0 from zeros
    y     = h @ Wo + bo                        # [B, T, O]

Sharding: data-parallel over batch (dim 0) across 8 NeuronCores; the time
recurrence stays local per core. Weights are replicated.

Per-core layout is "H-major": the hidden state lives as h^T tiles
[128 partitions = H-within-chunk, free = 8*m_chunk + batch], so the PSUM
output layout of one recurrence step IS the stationary-operand layout of
the next step - no transposes anywhere.

Everything on-device computes in bf16 inputs with fp32 PSUM accumulation
(max abs error vs the fp32 reference is ~1.4e-2 on output scale ~3.4).
"""

import os
import numpy as np
import ml_dtypes

import concourse.bass as bass
import concourse.mybir as mybir
import concourse.tile as tile
from concourse import bacc
from concourse.bass import ds
from concourse.bass_utils import run_bass_kernel_spmd

BF16 = mybir.dt.bfloat16
FP32 = mybir.dt.float32
bfnp = ml_dtypes.bfloat16

B, T, D, H, O = 64, 512, 256, 1024, 128
NCORES = 8
BL = B // NCORES          # 8 batch rows per core
SLOT = 8 * BL             # 64: one h/xproj timestep slot = 8 m-chunks x 8 batch
KH = H // 128             # 8 contraction chunks for Wh/Wo
MH = H // 128             # 8 output chunks of H
KD = D // 128             # 2 contraction chunks for Wx
NTOK = BL * T             # 4096 tokens per core
TCHUNK = 64               # timesteps per hardware-loop iteration

_cached_nc = None

# Results of the last run (for the local test harness; unused when grading).
LAST_RESULTS = None


def _build():
    nc = bacc.Bacc("TRN2", target_bir_lowering=False, debug=False)

    xT = nc.dram_tensor("xT", [D, NTOK], BF16, kind="ExternalInput")
    idn = nc.dram_tensor("idn", [128, 128], BF16, kind="ExternalInput")
    wh = nc.dram_tensor("wh", [H, H], BF16, kind="ExternalInput")
    wx = nc.dram_tensor("wx", [D, H], BF16, kind="ExternalInput")
    wo = nc.dram_tensor("wo", [H, O], BF16, kind="ExternalInput")
    bv = nc.dram_tensor("bv", [H], FP32, kind="ExternalInput")
    bov = nc.dram_tensor("bov", [O], FP32, kind="ExternalInput")
    yT = nc.dram_tensor("yT", [O, NTOK], FP32, kind="ExternalOutput")

    with tile.TileContext(nc) as tc:
        with (
            tc.tile_pool(name="const", bufs=1) as const,
            tc.tile_pool(name="scr", bufs=2) as scr,
            tc.tile_pool(name="yout", bufs=2) as yout,
        ):
            # Persistent SBUF residents.
            XP = const.tile([128, T * SLOT], BF16)       # xproj (phase-1 out)
            HS = const.tile([128, T * SLOT], BF16)       # h history (phase-3 in)
            WhS = const.tile([128, KH * H], BF16)        # Wh k-chunk k at [:, k*H:(k+1)*H]
            WxS = const.tile([128, KD * H], BF16)
            WoS = const.tile([128, KH * O], BF16)
            XTs = const.tile([128, KD * NTOK], BF16)
            bS = const.tile([128, MH], FP32)             # bS[p, m] = b[128m + p]
            boS = const.tile([128, 1], FP32)
            H0 = const.tile([128, SLOT], BF16)           # rotating h^T buffers
            H1 = const.tile([128, SLOT], BF16)
            IdS = const.tile([128, 128], BF16)           # identity, for xproj+=PSUM fold

            for k in range(KH):
                nc.sync.dma_start(WhS[:, k * H:(k + 1) * H], wh[k * 128:(k + 1) * 128, :])
            for k in range(KD):
                nc.sync.dma_start(WxS[:, k * H:(k + 1) * H], wx[k * 128:(k + 1) * 128, :])
                nc.sync.dma_start(XTs[:, k * NTOK:(k + 1) * NTOK], xT[k * 128:(k + 1) * 128, :])
            for k in range(KH):
                nc.sync.dma_start(WoS[:, k * O:(k + 1) * O], wo[k * 128:(k + 1) * 128, :])
            nc.sync.dma_start(IdS[:], idn[:, :])
            nc.sync.dma_start(bS[:], bv[:].rearrange("(m p) -> p m", p=128))
            nc.sync.dma_start(boS[:], bov[:].rearrange("(p one) -> p one", one=1))
            nc.vector.memset(H0[:], 0.0)

            XP3 = XP[:].rearrange("p (s f) -> p s f", f=SLOT)  # [128, T, SLOT]
            HS3 = HS[:].rearrange("p (s f) -> p s f", f=SLOT)

            # ---- Phase 1: xproj^T = Wx^T @ x^T + b, written into XH ----
            # Token tiles of 512 = 64 timesteps x 8 batch (t-major, b-minor).
            NT = 512
            with tc.tile_pool(name="p1psum", bufs=2, space="PSUM") as pp1:
                for nt in range(NTOK // NT):
                    for m in range(MH):
                        ps = pp1.tile([128, NT], FP32)
                        for k in range(KD):
                            nc.tensor.matmul(
                                ps[:],
                                WxS[:, k * H + 128 * m: k * H + 128 * (m + 1)],
                                XTs[:, k * NTOK + nt * NT: k * NTOK + (nt + 1) * NT],
                                start=(k == 0),
                                stop=(k == KD - 1),
                            )
                        dest = XP3[:, nt * 64:(nt + 1) * 64, 8 * m: 8 * (m + 1)]
                        nc.scalar.activation(
                            dest, ps[:],
                            mybir.ActivationFunctionType.Identity,
                            bias=bS[:, m:m + 1],
                        )

            # ---- Phase 2: the recurrence ----
            # Step s: PS[:, 8m:8m+8] = sum_k WhS_km^T @ Hprev[:, 8k:8k+8]
            #                          + I^T @ xproj_s[m]   (identity fold)
            #         Hcur = tanh(PS), in two halves so the next step's
            #         k<4 matmuls overlap the second half's tanh.
            # k-outer/m-inner ordering interleaves the 8 PSUM accumulation
            # groups (hardware handles this per-element; skip the checker).
            with tc.tile_pool(name="p2psum", bufs=2, space="PSUM") as pp2:
                def step(s_off, j):
                    hprev = H0 if j % 2 == 0 else H1
                    hcur = H1 if j % 2 == 0 else H0
                    ps = pp2.tile([128, SLOT], FP32, tag="ps2")
                    # One wide identity matmul resets all 8 groups to xproj_s
                    # (PSUM col layout 8m+b == XP slot layout). Reads only XP,
                    # so it has no dependency on the previous step and can
                    # issue during the inter-step tanh bubble.
                    nc.tensor.matmul(
                        ps[:],
                        IdS[:],
                        XP[:, ds(s_off + j * SLOT, SLOT)],
                        start=True,
                        stop=False,
                        skip_group_check=True,
                    )
                    for k in range(KH):
                        for m in range(MH):
                            nc.tensor.matmul(
                                ps[:, 8 * m: 8 * (m + 1)],
                                WhS[:, k * H + 128 * m: k * H + 128 * (m + 1)],
                                hprev[:, 8 * k: 8 * (k + 1)],
                                start=False,
                                stop=(k == KH - 1),
                                skip_group_check=True,
                            )
                    half = SLOT // 2
                    nc.scalar.activation(hcur[:, 0:half], ps[:, 0:half],
                                         mybir.ActivationFunctionType.Tanh)
                    nc.scalar.activation(hcur[:, half:SLOT], ps[:, half:SLOT],
                                         mybir.ActivationFunctionType.Tanh)
                    nc.vector.tensor_copy(HS[:, ds(s_off + j * SLOT, SLOT)], hcur[:])

                with tc.For_i(0, T * SLOT, TCHUNK * SLOT) as s_off:
                    for j in range(TCHUNK):
                        step(s_off, j)

            # ---- Phase 3: y^T = Wo^T @ h^T + bo ----
            with tc.tile_pool(name="p3psum", bufs=2, space="PSUM") as pp3:
                for nt in range(NTOK // NT):
                    ps = pp3.tile([128, NT], FP32)
                    for k in range(KH):
                        nc.tensor.matmul(
                            ps[:],
                            WoS[:, k * O:(k + 1) * O],
                            HS3[:, nt * 64:(nt + 1) * 64, 8 * k: 8 * (k + 1)],
                            start=(k == 0),
                            stop=(k == KH - 1),
                        )
                    yt = yout.tile([128, NT], FP32, tag="yt")
                    nc.scalar.activation(
                        yt[:], ps[:],
                        mybir.ActivationFunctionType.Identity,
                        bias=boS[:],
                    )
                    nc.sync.dma_start(yT[:, nt * NT:(nt + 1) * NT], yt[:])

    nc.compile()
    return nc


def _get_nc():
    global _cached_nc
    if _cached_nc is None:
        _cached_nc = _build()
    return _cached_nc


def kernel(inputs, Wx, Wh, b, Wo, bo):
    global LAST_RESULTS
    x = np.asarray(inputs, dtype=np.float32)        # [B, T, D]
    nc = _get_nc()

    xT_full = np.ascontiguousarray(x.transpose(2, 1, 0)).astype(bfnp)  # [D, T, B]
    whb = np.asarray(Wh, np.float32).astype(bfnp)
    wxb = np.asarray(Wx, np.float32).astype(bfnp)
    wob = np.asarray(Wo, np.float32).astype(bfnp)
    bf = np.ascontiguousarray(np.asarray(b, np.float32))
    bof = np.ascontiguousarray(np.asarray(bo, np.float32))

    in_maps = []
    for c in range(NCORES):
        xs = np.ascontiguousarray(xT_full[:, :, c * BL:(c + 1) * BL]).reshape(D, NTOK)
        in_maps.append({
            "xT": xs, "wh": whb, "wx": wxb, "wo": wob, "bv": bf, "bov": bof,
            "idn": np.eye(128, dtype=np.float32).astype(bfnp),
        })

    res = run_bass_kernel_spmd(nc, in_maps, list(range(NCORES)))
    LAST_RESULTS = res

    y = np.empty((B, T, O), np.float32)
    for c in range(NCORES):
        ytc = res.results[c]["yT"]                   # [O, T*BL], col = t*BL + b
        y[c * BL:(c + 1) * BL] = ytc.reshape(O, T, BL).transpose(2, 1, 0)
    return y



# revision 2
# speedup vs baseline: 1.0950x; 1.0950x over previous
"""Trainium2 Bass kernel for a Keras SimpleRNN wrapper (raw-bass v2).

    xproj = inputs @ Wx + b                    # [B, T, H]
    h_t   = tanh(xproj_t + h_{t-1} @ Wh)       # scan over T, h_0 = 0
    y     = h @ Wo + bo                        # [B, T, O]

Data-parallel over batch (8 rows/core), recurrence local per core —
same sharding as the baseline, but the step pipeline is restructured:

The baseline's per-step cost was 65 MM-pairs at the ~26.5ns issue floor
(1.72us) + a ~0.97us serial tanh bubble (PSUM drain -> Act -> sem ->
next step). v2 splits each step's H output into two PSUM groups
(A = m0..3, B = m4..7). tanh(A) fires while the B half of the stream
is still issuing, and the next step's k0..3 chunks gate only on
tanh(A), k4..7 + B on tanh(B) - so the Act work hides almost entirely
under the MM stream. tanh also writes the h history (HS) directly
(dynamic APs), removing the DVE copy from the critical chain.
Everything is raw bass (no TileContext): no per-iteration loop
barriers or semaphore-reset stalls.
"""

import numpy as np
import ml_dtypes

import concourse.bass as bass
import concourse.mybir as mybir
from concourse import bacc
from concourse.bass import ds
from concourse.bass import MonotonicSemaphore
from concourse.bass_utils import run_bass_kernel_spmd

BF16 = mybir.dt.bfloat16
FP32 = mybir.dt.float32
bfnp = ml_dtypes.bfloat16

B, T, D, H, O = 64, 512, 256, 1024, 128
NCORES = 8
BL = B // NCORES          # 8 batch rows per core
SLOT = 8 * BL             # 64 cols: one timestep slot = 8 m-chunks x 8 batch
KH = H // 128             # 8 contraction chunks for Wh/Wo
MH = H // 128             # 8 output chunks of H
KD = D // 128             # 2 contraction chunks for Wx
NTOK = BL * T             # 4096 tokens per core
NT = 512                  # tokens per phase-1/3 PSUM tile (64 steps x 8 batch)
NBLK = T // 64            # 8 outer blocks (y computed per block)

_cached_nc = None
LAST_RESULTS = None


def _build():
    nc = bacc.Bacc("TRN2", target_bir_lowering=False, debug=False)

    xT = nc.dram_tensor("xT", [D, NTOK], BF16, kind="ExternalInput")
    idn = nc.dram_tensor("idn", [128, 128], BF16, kind="ExternalInput")
    wh = nc.dram_tensor("wh", [H, H], BF16, kind="ExternalInput")
    wx = nc.dram_tensor("wx", [D, H], BF16, kind="ExternalInput")
    wo = nc.dram_tensor("wo", [H, O], BF16, kind="ExternalInput")
    bv = nc.dram_tensor("bv", [H], FP32, kind="ExternalInput")
    bov = nc.dram_tensor("bov", [O], FP32, kind="ExternalInput")
    yT = nc.dram_tensor("yT", [O, NTOK], FP32, kind="ExternalOutput")
    ymark = nc.dram_tensor("ymark", [128, 1], FP32, kind="Internal")

    # ---- SBUF (raw; bytes per partition in comments) ----
    XP = nc.alloc_sbuf_tensor("XP", [128, T * SLOT], BF16)        # 64K
    HS = nc.alloc_sbuf_tensor("HS", [128, (T + 1) * SLOT], BF16)  # 64K+128
    XTs = nc.alloc_sbuf_tensor("XTs", [128, KD * NTOK], BF16)     # 16K
    WhS = nc.alloc_sbuf_tensor("WhS", [128, KH * H], BF16)        # 16K
    WxS = nc.alloc_sbuf_tensor("WxS", [128, KD * H], BF16)        # 4K
    WoS = nc.alloc_sbuf_tensor("WoS", [128, KH * O], BF16)        # 2K
    bS = nc.alloc_sbuf_tensor("bS", [128, MH], FP32)              # 32
    boS = nc.alloc_sbuf_tensor("boS", [128, 1], FP32)             # 4
    IdS = nc.alloc_sbuf_tensor("IdS", [128, 128], BF16)           # 256
    YTs = nc.alloc_sbuf_tensor("YTs", [128, 2 * NT], FP32)        # 4K

    # ---- PSUM ----
    P1 = [nc.alloc_psum_tensor(f"P1_{i}", [128, NT], FP32) for i in range(2)]
    PA = [nc.alloc_psum_tensor(f"PA_{i}", [128, SLOT // 2], FP32) for i in range(2)]
    PB = [nc.alloc_psum_tensor(f"PB_{i}", [128, SLOT // 2], FP32) for i in range(2)]

    # ---- semaphores ----
    WS = nc.alloc_semaphore("ws_v2")    # init DMAs + HS memset
    P1S = nc.alloc_semaphore("p1s_v2")  # phase1 PSUM ready     +1/tile
    A1S = nc.alloc_semaphore("a1s_v2")  # phase1 act done       +1/tile
    ZA = nc.alloc_semaphore("za_v2")    # step group-A PSUM ready  +1/step
    ZB = nc.alloc_semaphore("zb_v2")    # step group-B PSUM ready  +1/step
    TA = nc.alloc_semaphore("ta_v2")    # tanh A done           +1/step
    TB = nc.alloc_semaphore("tb_v2")    # tanh B done           +1/step
    P3S = nc.alloc_semaphore("p3s_v2")  # y PSUM ready          +1/block
    YAS = nc.alloc_semaphore("yas_v2")  # y act done            +1/block
    YDS = nc.alloc_semaphore("yds_v2")  # y marker DMA done     +16/block
    YJS = nc.alloc_semaphore("yjs_v2")  # y data DMA raw ticks (not waited)

    for s in (WS, P1S, A1S, ZA, ZB, TA, TB, P3S, YAS, YDS, YJS):
        nc.gpsimd.sem_clear(s)
    nc.all_engine_barrier()

    # ---- init loads ----
    nw = 0
    for k in range(KH):
        nc.sync.dma_start(WhS[:, k * H:(k + 1) * H],
                          wh[k * 128:(k + 1) * 128, :]).then_inc(WS, 16)
        nw += 1
    for k in range(KD):
        nc.sync.dma_start(WxS[:, k * H:(k + 1) * H],
                          wx[k * 128:(k + 1) * 128, :]).then_inc(WS, 16)
        nw += 1
        nc.sync.dma_start(XTs[:, k * NTOK:(k + 1) * NTOK],
                          xT[k * 128:(k + 1) * 128, :]).then_inc(WS, 16)
        nw += 1
    for k in range(KH):
        nc.sync.dma_start(WoS[:, k * O:(k + 1) * O],
                          wo[k * 128:(k + 1) * 128, :]).then_inc(WS, 16)
        nw += 1
    nc.sync.dma_start(IdS[:], idn[:, :]).then_inc(WS, 16)
    with nc.allow_non_contiguous_dma(reason="tiny bias vectors"):
        nc.sync.dma_start(bS[:], bv[:].rearrange("(m p) -> p m", p=128)).then_inc(WS, 16)
        nc.sync.dma_start(boS[:], bov[:].rearrange("(p one) -> p one", one=1)).then_inc(WS, 16)
    nw += 3
    nc.vector.memset(HS[:, 0:SLOT], 0.0).then_inc(WS, 16)
    nw += 1
    WTARGET = 16 * nw

    nc.tensor.wait_ge(WS, WTARGET)
    nc.scalar.wait_ge(WS, WTARGET)

    XP3 = XP[:].rearrange("p (s f) -> p s f", f=SLOT)   # [128, T, 64]
    HS3 = HS[:].rearrange("p (s f) -> p s f", f=SLOT)   # [128, T+1, 64]

    # ---- phase 1: xproj^T = Wx^T @ x^T + b ----
    # 64 (nt, m) pairs; PSUM ping-pong; Act writes bf16 XP with bias.
    a1_pe = MonotonicSemaphore(nc.tensor, A1S)
    pair = 0
    for nt in range(NTOK // NT):
        for m in range(MH):
            pp = pair % 2
            if pair >= 2:
                a1_pe.wait_inc(1)      # act of pair-2 done -> PSUM free
            for k in range(KD):
                mm = nc.tensor.matmul(
                    P1[pp][:],
                    WxS[:, k * H + 128 * m: k * H + 128 * (m + 1)],
                    XTs[:, k * NTOK + nt * NT: k * NTOK + (nt + 1) * NT],
                    start=(k == 0),
                    stop=(k == KD - 1),
                )
                if k == KD - 1:
                    mm.then_inc(P1S, 1)
            pair += 1
    # Act side of phase 1
    p1_act = MonotonicSemaphore(nc.scalar, P1S)
    pair = 0
    for nt in range(NTOK // NT):
        for m in range(MH):
            pp = pair % 2
            p1_act.wait_inc(1)
            nc.scalar.activation(
                XP3[:, nt * 64:(nt + 1) * 64, 8 * m: 8 * (m + 1)],
                P1[pp][:],
                mybir.ActivationFunctionType.Identity,
                bias=bS[:, m:m + 1],
            ).then_inc(A1S, 1)
            pair += 1

    # PE: remaining phase-1 acts must finish before recurrence PSUM overlap
    # is irrelevant (different PSUM tensors) - but step MMs read XP: gate once.
    a1_pe.wait_inc(2)                 # waits A1S >= 64 (all XP written)

    # ---- recurrence ----
    # step t: PA[j] cols 8m+b (m 0..3) = group A, PB[j] = group B (m 4..7).
    # Gates: A:k0-3 tiles wait TA>=t; A:k4-7, B tiles wait TB>=t.
    # tanh A fires after ZA (mid-stream), tanh B after ZB (stream end).
    rta = nc.tensor.alloc_register("rta")
    nc.tensor.reg_mov(rta, 0)
    za_act = MonotonicSemaphore(nc.scalar, ZA)
    zb_act = MonotonicSemaphore(nc.scalar, ZB)
    yd_act = MonotonicSemaphore(nc.scalar, YDS)
    p3_act = MonotonicSemaphore(nc.scalar, P3S)
    ya_pe = MonotonicSemaphore(nc.tensor, YAS)

    def step(blk, iv, j):
        # t = blk*64 + iv*2 + j; read h_{t-1} at HS slot t, write slot t+1
        rd = iv * (2 * SLOT) + (blk * 64 + j) * SLOT
        wr = iv * (2 * SLOT) + (blk * 64 + j + 1) * SLOT
        xo = iv * (2 * SLOT) + (blk * 64 + j) * SLOT
        # identity MMs open both PSUM groups with xproj (no h dependency)
        nc.tensor.matmul(PA[j][:], IdS[:], XP[:, ds(xo, 32)],
                         start=True, stop=False, skip_group_check=True)
        nc.tensor.matmul(PB[j][:], IdS[:], XP[:, ds(xo + 32, 32)],
                         start=True, stop=False, skip_group_check=True)
        nc.tensor.wait_ge(TA, rta)
        for k in range(KH):
            if k == 4:
                nc.tensor.wait_ge(TB, rta)
            for m in range(4):
                mm = nc.tensor.matmul(
                    PA[j][:, 8 * m: 8 * (m + 1)],
                    WhS[:, k * H + 128 * m: k * H + 128 * (m + 1)],
                    HS[:, ds(rd + k * 8, 8)],
                    start=False, stop=(k == KH - 1),
                    skip_group_check=True,
                )
                if k == KH - 1 and m == 3:
                    mm.then_inc(ZA, 1)
        for k in range(KH):
            for m in range(4, 8):
                mm = nc.tensor.matmul(
                    PB[j][:, 8 * (m - 4): 8 * (m - 3)],
                    WhS[:, k * H + 128 * m: k * H + 128 * (m + 1)],
                    HS[:, ds(rd + k * 8, 8)],
                    start=False, stop=(k == KH - 1),
                    skip_group_check=True,
                )
                if k == KH - 1 and m == 7:
                    mm.then_inc(ZB, 1)
        nc.tensor.reg_add(rta, rta, 1)
        # Act: tanh halves -> HS slot t+1
        za_act.wait_inc(1)
        nc.scalar.activation(HS[:, ds(wr, 32)], PA[j][:],
                             mybir.ActivationFunctionType.Tanh).then_inc(TA, 1)
        zb_act.wait_inc(1)
        nc.scalar.activation(HS[:, ds(wr + 32, 32)], PB[j][:],
                             mybir.ActivationFunctionType.Tanh).then_inc(TB, 1)

    for blk in range(NBLK):
        with nc.Fori(0, 32) as iv:
            step(blk, iv, 0)
            step(blk, iv, 1)

        # ---- inline y for this block: steps blk*64 .. blk*64+63 ----
        # rhs chunk k: HS slots blk*64+1 .. blk*64+64, cols 8k..8k+8
        pp = blk % 2
        nc.tensor.wait_ge(TB, rta)     # all tanh of this block written to HS
        if blk >= 2:
            ya_pe.wait_inc(1)          # y-act of blk-2 done -> P1[pp] free
        for k in range(KH):
            mm = nc.tensor.matmul(
                P1[pp][:],
                WoS[:, k * O:(k + 1) * O],
                HS3[:, blk * 64 + 1:blk * 64 + 65, 8 * k:8 * (k + 1)],
                start=(k == 0), stop=(k == KH - 1),
            )
            if k == KH - 1:
                mm.then_inc(P3S, 1)
        # Act: bias add -> YTs ping-pong; SP: DMA out
        p3_act.wait_inc(1)
        if blk >= 2:
            # wait for ALL markers through blk-1 (engine completion order
            # across blocks isn't guaranteed; blk-1's marker <- act blk-1
            # which precedes this act, so never circular)
            yd_act.wait_inc(32 if blk == 2 else 16)
        nc.scalar.activation(YTs[:, pp * NT:(pp + 1) * NT], P1[pp][:],
                             mybir.ActivationFunctionType.Identity,
                             bias=boS[:]).then_inc(YAS, 1)
        nc.sync.wait_ge(YAS, blk + 1)
        nc.sync.dma_start(yT[:, blk * NT:(blk + 1) * NT],
                          YTs[:, pp * NT:(pp + 1) * NT]).then_inc(YJS, 16)
        # same-queue marker: lands strictly after the block's y data
        nc.sync.dma_start(ymark[:, :], YTs[:, pp * NT:pp * NT + 1]) \
            .then_inc(YDS, 16)

    # ---- drain: final y DMAs complete before kernel end ----
    nc.gpsimd.wait_ge(YDS, 16 * NBLK)

    nc.compile()
    return nc


def _get_nc():
    global _cached_nc
    if _cached_nc is None:
        _cached_nc = _build()
    return _cached_nc


def kernel(inputs, Wx, Wh, b, Wo, bo):
    global LAST_RESULTS
    x = np.asarray(inputs, dtype=np.float32)        # [B, T, D]
    nc = _get_nc()

    xT_full = np.ascontiguousarray(x.transpose(2, 1, 0)).astype(bfnp)  # [D, T, B]
    whb = np.asarray(Wh, np.float32).astype(bfnp)
    wxb = np.asarray(Wx, np.float32).astype(bfnp)
    wob = np.asarray(Wo, np.float32).astype(bfnp)
    bf = np.ascontiguousarray(np.asarray(b, np.float32))
    bof = np.ascontiguousarray(np.asarray(bo, np.float32))

    in_maps = []
    for c in range(NCORES):
        xs = np.ascontiguousarray(xT_full[:, :, c * BL:(c + 1) * BL]).reshape(D, NTOK)
        in_maps.append({
            "xT": xs, "wh": whb, "wx": wxb, "wo": wob, "bv": bf, "bov": bof,
            "idn": np.eye(128, dtype=np.float32).astype(bfnp),
        })

    res = run_bass_kernel_spmd(nc, in_maps, list(range(NCORES)))
    LAST_RESULTS = res

    y = np.empty((B, T, O), np.float32)
    for c in range(NCORES):
        ytc = res.results[c]["yT"]                   # [O, T*BL], col = t*BL + b
        y[c * BL:(c + 1) * BL] = ytc.reshape(O, T, BL).transpose(2, 1, 0)
    return y


# revision 5
# speedup vs baseline: 1.0970x; 1.0018x over previous
"""Trainium2 Bass kernel for a Keras SimpleRNN wrapper (raw-bass v2).

    xproj = inputs @ Wx + b                    # [B, T, H]
    h_t   = tanh(xproj_t + h_{t-1} @ Wh)       # scan over T, h_0 = 0
    y     = h @ Wo + bo                        # [B, T, O]

Data-parallel over batch (8 rows/core), recurrence local per core —
same sharding as the baseline, but the step pipeline is restructured:

The baseline's per-step cost was 65 MM-pairs at the ~26.5ns issue floor
(1.72us) + a ~0.97us serial tanh bubble (PSUM drain -> Act -> sem ->
next step). v2 splits each step's H output into two PSUM groups
(A = m0..3, B = m4..7). tanh(A) fires while the B half of the stream
is still issuing, and the next step's k0..3 chunks gate only on
tanh(A), k4..7 + B on tanh(B) - so the Act work hides almost entirely
under the MM stream. tanh also writes the h history (HS) directly
(dynamic APs), removing the DVE copy from the critical chain.
Everything is raw bass (no TileContext): no per-iteration loop
barriers or semaphore-reset stalls.
"""

import numpy as np
import ml_dtypes

import concourse.bass as bass
import concourse.mybir as mybir
from concourse import bacc
from concourse.bass import ds
from concourse.bass import MonotonicSemaphore
from concourse.bass_utils import run_bass_kernel_spmd

BF16 = mybir.dt.bfloat16
FP32 = mybir.dt.float32
bfnp = ml_dtypes.bfloat16

B, T, D, H, O = 64, 512, 256, 1024, 128
NCORES = 8
BL = B // NCORES          # 8 batch rows per core
SLOT = 8 * BL             # 64 cols: one timestep slot = 8 m-chunks x 8 batch
KH = H // 128             # 8 contraction chunks for Wh/Wo
MH = H // 128             # 8 output chunks of H
KD = D // 128             # 2 contraction chunks for Wx
NTOK = BL * T             # 4096 tokens per core
NT = 512                  # tokens per phase-1/3 PSUM tile (64 steps x 8 batch)
NBLK = T // 64            # 8 outer blocks (y computed per block)

_cached_nc = None
LAST_RESULTS = None


def _build():
    nc = bacc.Bacc("TRN2", target_bir_lowering=False, debug=False)

    xT = nc.dram_tensor("xT", [D, NTOK], BF16, kind="ExternalInput")
    idn = nc.dram_tensor("idn", [128, 128], BF16, kind="ExternalInput")
    wh = nc.dram_tensor("wh", [H, H], BF16, kind="ExternalInput")
    wx = nc.dram_tensor("wx", [D, H], BF16, kind="ExternalInput")
    wo = nc.dram_tensor("wo", [H, O], BF16, kind="ExternalInput")
    bv = nc.dram_tensor("bv", [H], FP32, kind="ExternalInput")
    bov = nc.dram_tensor("bov", [O], FP32, kind="ExternalInput")
    yT = nc.dram_tensor("yT", [O, NTOK], FP32, kind="ExternalOutput")
    ymark = nc.dram_tensor("ymark", [128, 1], FP32, kind="Internal")

    # ---- SBUF (raw; bytes per partition in comments) ----
    # padded by 2 slots: ds() range checks use the loop bound (end-1), not
    # end-step, so the conservative max overshoots by one iteration
    XP = nc.alloc_sbuf_tensor("XP", [128, (T + 2) * SLOT], BF16)  # 64K
    HS = nc.alloc_sbuf_tensor("HS", [128, (T + 3) * SLOT], BF16)  # 64K+
    XTs = nc.alloc_sbuf_tensor("XTs", [128, KD * NTOK], BF16)     # 16K
    WhS = nc.alloc_sbuf_tensor("WhS", [128, KH * H], BF16)        # 16K
    WxS = nc.alloc_sbuf_tensor("WxS", [128, KD * H], BF16)        # 4K
    WoS = nc.alloc_sbuf_tensor("WoS", [128, KH * O], BF16)        # 2K
    bS = nc.alloc_sbuf_tensor("bS", [128, MH], FP32)              # 32
    boS = nc.alloc_sbuf_tensor("boS", [128, 1], FP32)             # 4
    IdS = nc.alloc_sbuf_tensor("IdS", [128, 128], BF16)           # 256
    YTs = nc.alloc_sbuf_tensor("YTs", [128, 2 * NT], FP32)        # 4K
    # k-major h copy, 2-block ring: HK[:, k*1024 + (blk%2)*512 + (t%64)*8 + b]
    HK = nc.alloc_sbuf_tensor("HK", [128, KH * 2 * NT], BF16)     # 16K

    # ---- PSUM ----
    P1 = [nc.alloc_psum_tensor(f"P1_{i}", [128, NT], FP32) for i in range(2)]
    PA = [nc.alloc_psum_tensor(f"PA_{i}", [128, SLOT // 2], FP32) for i in range(2)]
    PB = [nc.alloc_psum_tensor(f"PB_{i}", [128, SLOT // 2], FP32) for i in range(2)]

    # ---- semaphores ----
    WS = nc.alloc_semaphore("ws_v2")    # init DMAs + HS memset
    P1S = nc.alloc_semaphore("p1s_v2")  # phase1 PSUM ready     +1/tile
    A1S = nc.alloc_semaphore("a1s_v2")  # phase1 act done       +1/tile
    ZA = nc.alloc_semaphore("za_v2")    # step group-A PSUM ready  +1/step
    ZB = nc.alloc_semaphore("zb_v2")    # step group-B PSUM ready  +1/step
    TA = nc.alloc_semaphore("ta_v2")    # tanh A done           +1/step
    TB = nc.alloc_semaphore("tb_v2")    # tanh B done           +1/step
    P3S = nc.alloc_semaphore("p3s_v2")  # y PSUM ready          +1/block
    YAS = nc.alloc_semaphore("yas_v2")  # y act done            +1/block
    YDS = nc.alloc_semaphore("yds_v2")  # y marker DMA done     +16/block
    YJS = nc.alloc_semaphore("yjs_v2")  # y data DMA raw ticks (not waited)
    CS = nc.alloc_semaphore("cs_v2")    # DVE HK copy done      +1/step

    for s in (WS, P1S, A1S, ZA, ZB, TA, TB, P3S, YAS, YDS, YJS, CS):
        nc.gpsimd.sem_clear(s)
    nc.all_engine_barrier()

    # ---- init loads ----
    nw = 0
    for k in range(KH):
        nc.sync.dma_start(WhS[:, k * H:(k + 1) * H],
                          wh[k * 128:(k + 1) * 128, :]).then_inc(WS, 16)
        nw += 1
    for k in range(KD):
        nc.sync.dma_start(WxS[:, k * H:(k + 1) * H],
                          wx[k * 128:(k + 1) * 128, :]).then_inc(WS, 16)
        nw += 1
        nc.sync.dma_start(XTs[:, k * NTOK:(k + 1) * NTOK],
                          xT[k * 128:(k + 1) * 128, :]).then_inc(WS, 16)
        nw += 1
    for k in range(KH):
        nc.sync.dma_start(WoS[:, k * O:(k + 1) * O],
                          wo[k * 128:(k + 1) * 128, :]).then_inc(WS, 16)
        nw += 1
    nc.sync.dma_start(IdS[:], idn[:, :]).then_inc(WS, 16)
    with nc.allow_non_contiguous_dma(reason="tiny bias vectors"):
        nc.sync.dma_start(bS[:], bv[:].rearrange("(m p) -> p m", p=128)).then_inc(WS, 16)
        nc.sync.dma_start(boS[:], bov[:].rearrange("(p one) -> p one", one=1)).then_inc(WS, 16)
    nw += 3
    nc.vector.memset(HS[:, 0:SLOT], 0.0).then_inc(WS, 16)
    nw += 1
    WTARGET = 16 * nw

    nc.tensor.wait_ge(WS, WTARGET)
    nc.scalar.wait_ge(WS, WTARGET)

    XP3 = XP[:].rearrange("p (s f) -> p s f", f=SLOT)   # [128, T, 64]
    HS3 = HS[:].rearrange("p (s f) -> p s f", f=SLOT)   # [128, T+1, 64]

    # ---- phase 1: xproj^T = Wx^T @ x^T + b ----
    # 64 (nt, m) pairs; PSUM ping-pong; Act writes bf16 XP with bias.
    a1_pe = MonotonicSemaphore(nc.tensor, A1S)
    pair = 0
    for nt in range(NTOK // NT):
        for m in range(MH):
            pp = pair % 2
            if pair >= 2:
                a1_pe.wait_inc(1)      # act of pair-2 done -> PSUM free
            for k in range(KD):
                mm = nc.tensor.matmul(
                    P1[pp][:],
                    WxS[:, k * H + 128 * m: k * H + 128 * (m + 1)],
                    XTs[:, k * NTOK + nt * NT: k * NTOK + (nt + 1) * NT],
                    start=(k == 0),
                    stop=(k == KD - 1),
                )
                if k == KD - 1:
                    mm.then_inc(P1S, 1)
            pair += 1
    # Act side of phase 1
    p1_act = MonotonicSemaphore(nc.scalar, P1S)
    pair = 0
    for nt in range(NTOK // NT):
        for m in range(MH):
            pp = pair % 2
            p1_act.wait_inc(1)
            nc.scalar.activation(
                XP3[:, nt * 64:(nt + 1) * 64, 8 * m: 8 * (m + 1)],
                P1[pp][:],
                mybir.ActivationFunctionType.Identity,
                bias=bS[:, m:m + 1],
            ).then_inc(A1S, 1)
            pair += 1

    # PE: remaining phase-1 acts must finish before recurrence PSUM overlap
    # is irrelevant (different PSUM tensors) - but step MMs read XP: gate once.
    a1_pe.wait_inc(2)                 # waits A1S >= 64 (all XP written)

    # ---- recurrence ----
    # step t: PA[j] cols 8m+b (m 0..3) = group A, PB[j] = group B (m 4..7).
    # Gates: A:k0-3 tiles wait TA>=t; A:k4-7, B tiles wait TB>=t.
    # tanh A fires after ZA (mid-stream), tanh B after ZB (stream end).
    rta = nc.tensor.alloc_register("rta")
    nc.tensor.reg_mov(rta, 0)
    za_act = MonotonicSemaphore(nc.scalar, ZA)
    zb_act = MonotonicSemaphore(nc.scalar, ZB)
    yd_act = MonotonicSemaphore(nc.scalar, YDS)
    p3_act = MonotonicSemaphore(nc.scalar, P3S)
    ya_pe = MonotonicSemaphore(nc.tensor, YAS)
    cs_pe = MonotonicSemaphore(nc.tensor, CS)
    tb_dve = MonotonicSemaphore(nc.vector, TB)
    p3_dve = MonotonicSemaphore(nc.vector, P3S)
    HKr = HK[:].rearrange("p (k s) -> p k s", k=KH)   # [128, 8, 1024]

    def step(blk, iv, rk, j):
        # iv steps by 2*SLOT per body (column units); rk steps by 16 (HK cols)
        # t = blk*64 + pair*2 + j
        rd = iv + (blk * 64 + j) * SLOT
        wr = iv + (blk * 64 + j + 1) * SLOT
        xo = iv + (blk * 64 + j) * SLOT
        # identity MMs open both PSUM groups with xproj (no h dependency)
        nc.tensor.matmul(PA[j][:], IdS[:], XP[:, ds(xo, 32)],
                         start=True, stop=False, skip_group_check=True)
        nc.tensor.matmul(PB[j][:], IdS[:], XP[:, ds(xo + 32, 32)],
                         start=True, stop=False, skip_group_check=True)
        nc.tensor.wait_ge(TA, rta)
        for k in range(KH):
            if k == 4:
                nc.tensor.wait_ge(TB, rta)
            for m in range(4):
                mm = nc.tensor.matmul(
                    PA[j][:, 8 * m: 8 * (m + 1)],
                    WhS[:, k * H + 128 * m: k * H + 128 * (m + 1)],
                    HS[:, ds(rd + k * 8, 8)],
                    start=False, stop=(k == KH - 1),
                    skip_group_check=True,
                )
                if k == KH - 1 and m == 3:
                    mm.then_inc(ZA, 1)
        for k in range(KH):
            for m in range(4, 8):
                mm = nc.tensor.matmul(
                    PB[j][:, 8 * (m - 4): 8 * (m - 3)],
                    WhS[:, k * H + 128 * m: k * H + 128 * (m + 1)],
                    HS[:, ds(rd + k * 8, 8)],
                    start=False, stop=(k == KH - 1),
                    skip_group_check=True,
                )
                if k == KH - 1 and m == 7:
                    mm.then_inc(ZB, 1)
        nc.tensor.reg_add(rta, rta, 1)
        # Act: tanh halves -> HS slot t+1
        za_act.wait_inc(1)
        nc.scalar.activation(HS[:, ds(wr, 32)], PA[j][:],
                             mybir.ActivationFunctionType.Tanh).then_inc(TA, 1)
        zb_act.wait_inc(1)
        nc.scalar.activation(HS[:, ds(wr + 32, 32)], PB[j][:],
                             mybir.ActivationFunctionType.Tanh).then_inc(TB, 1)

    for blk in range(NBLK):
        with nc.Fori(0, 64 * SLOT, 2 * SLOT) as iv:
            step(blk, iv, None, 0)
            step(blk, iv, None, 1)

        # ---- inline y for this block: steps blk*64 .. blk*64+63 ----
        # rhs chunk k: HS slots blk*64+1 .. blk*64+64, cols 8k..8k+8
        pp = blk % 2
        nc.tensor.wait_ge(TB, rta)     # all tanh of this block written to HS
        if blk >= 2:
            ya_pe.wait_inc(1)          # y-act of blk-2 done -> P1[pp] free
        for k in range(KH):
            mm = nc.tensor.matmul(
                P1[pp][:],
                WoS[:, k * O:(k + 1) * O],
                HS3[:, blk * 64 + 1:blk * 64 + 65, 8 * k:8 * (k + 1)],
                start=(k == 0), stop=(k == KH - 1),
            )
            if k == KH - 1:
                mm.then_inc(P3S, 1)
        # Act: bias add -> YTs ping-pong; SP: DMA out
        p3_act.wait_inc(1)
        if blk >= 2:
            # wait for ALL markers through blk-1 (engine completion order
            # across blocks isn't guaranteed; blk-1's marker <- act blk-1
            # which precedes this act, so never circular)
            yd_act.wait_inc(32 if blk == 2 else 16)
        nc.scalar.activation(YTs[:, pp * NT:(pp + 1) * NT], P1[pp][:],
                             mybir.ActivationFunctionType.Identity,
                             bias=boS[:]).then_inc(YAS, 1)
        nc.sync.wait_ge(YAS, blk + 1)
        nc.sync.dma_start(yT[:, blk * NT:(blk + 1) * NT],
                          YTs[:, pp * NT:(pp + 1) * NT]).then_inc(YJS, 16)
        # same-queue marker: lands strictly after the block's y data
        nc.sync.dma_start(ymark[:, :], YTs[:, pp * NT:pp * NT + 1]) \
            .then_inc(YDS, 16)

    # ---- drain: final y DMAs complete before kernel end ----
    nc.gpsimd.wait_ge(YDS, 16 * NBLK)

    nc.compile()
    return nc


def _get_nc():
    global _cached_nc
    if _cached_nc is None:
        _cached_nc = _build()
    return _cached_nc


def kernel(inputs, Wx, Wh, b, Wo, bo):
    global LAST_RESULTS
    x = np.asarray(inputs, dtype=np.float32)        # [B, T, D]
    nc = _get_nc()

    xT_full = np.ascontiguousarray(x.transpose(2, 1, 0)).astype(bfnp)  # [D, T, B]
    whb = np.asarray(Wh, np.float32).astype(bfnp)
    wxb = np.asarray(Wx, np.float32).astype(bfnp)
    wob = np.asarray(Wo, np.float32).astype(bfnp)
    bf = np.ascontiguousarray(np.asarray(b, np.float32))
    bof = np.ascontiguousarray(np.asarray(bo, np.float32))

    in_maps = []
    for c in range(NCORES):
        xs = np.ascontiguousarray(xT_full[:, :, c * BL:(c + 1) * BL]).reshape(D, NTOK)
        in_maps.append({
            "xT": xs, "wh": whb, "wx": wxb, "wo": wob, "bv": bf, "bov": bof,
            "idn": np.eye(128, dtype=np.float32).astype(bfnp),
        })

    res = run_bass_kernel_spmd(nc, in_maps, list(range(NCORES)))
    LAST_RESULTS = res

    y = np.empty((B, T, O), np.float32)
    for c in range(NCORES):
        ytc = res.results[c]["yT"]                   # [O, T*BL], col = t*BL + b
        y[c * BL:(c + 1) * BL] = ytc.reshape(O, T, BL).transpose(2, 1, 0)
    return y


# revision 6
# speedup vs baseline: 1.3194x; 1.2027x over previous
"""Trainium2 Bass kernel for a Keras SimpleRNN wrapper (raw-bass v2).

    xproj = inputs @ Wx + b                    # [B, T, H]
    h_t   = tanh(xproj_t + h_{t-1} @ Wh)       # scan over T, h_0 = 0
    y     = h @ Wo + bo                        # [B, T, O]

Data-parallel over batch (8 rows/core), recurrence local per core —
same sharding as the baseline, but the step pipeline is restructured:

The baseline's per-step cost was 65 MM-pairs at the ~26.5ns issue floor
(1.72us) + a ~0.97us serial tanh bubble (PSUM drain -> Act -> sem ->
next step). v2 splits each step's H output into two PSUM groups
(A = m0..3, B = m4..7). tanh(A) fires while the B half of the stream
is still issuing, and the next step's k0..3 chunks gate only on
tanh(A), k4..7 + B on tanh(B) - so the Act work hides almost entirely
under the MM stream. tanh also writes the h history (HS) directly
(dynamic APs), removing the DVE copy from the critical chain.
Everything is raw bass (no TileContext): no per-iteration loop
barriers or semaphore-reset stalls.
"""

import numpy as np
import ml_dtypes

import concourse.bass as bass
import concourse.mybir as mybir
from concourse import bacc
from concourse.bass import ds
from concourse.bass import MonotonicSemaphore
from concourse.bass_utils import run_bass_kernel_spmd

BF16 = mybir.dt.bfloat16
FP32 = mybir.dt.float32
bfnp = ml_dtypes.bfloat16

B, T, D, H, O = 64, 512, 256, 1024, 128
NCORES = 8
BL = B // NCORES          # 8 batch rows per core
SLOT = 8 * BL             # 64 cols: one timestep slot = 8 m-chunks x 8 batch
KH = H // 128             # 8 contraction chunks for Wh/Wo
MH = H // 128             # 8 output chunks of H
KD = D // 128             # 2 contraction chunks for Wx
NTOK = BL * T             # 4096 tokens per core
NT = 512                  # tokens per phase-1/3 PSUM tile (64 steps x 8 batch)
NBLK = T // 64            # 8 outer blocks (y computed per block)

_cached_nc = None
LAST_RESULTS = None


def _build():
    nc = bacc.Bacc("TRN2", target_bir_lowering=False, debug=False)

    xT = nc.dram_tensor("xT", [D, NTOK], BF16, kind="ExternalInput")
    idn = nc.dram_tensor("idn", [128, 128], BF16, kind="ExternalInput")
    wh = nc.dram_tensor("wh", [H, H], BF16, kind="ExternalInput")
    wx = nc.dram_tensor("wx", [D, H], BF16, kind="ExternalInput")
    wo = nc.dram_tensor("wo", [H, O], BF16, kind="ExternalInput")
    bv = nc.dram_tensor("bv", [H], FP32, kind="ExternalInput")
    bov = nc.dram_tensor("bov", [O], FP32, kind="ExternalInput")
    yT = nc.dram_tensor("yT", [O, NTOK], FP32, kind="ExternalOutput")
    ymark = nc.dram_tensor("ymark", [128, 1], FP32, kind="Internal")

    # ---- SBUF (raw; bytes per partition in comments) ----
    # padded by 2 slots: ds() range checks use the loop bound (end-1), not
    # end-step, so the conservative max overshoots by one iteration
    XP = nc.alloc_sbuf_tensor("XP", [128, (T + 8) * SLOT], BF16)  # 64K+pad
    H0 = nc.alloc_sbuf_tensor("H0", [128, SLOT], BF16)
    H1 = nc.alloc_sbuf_tensor("H1", [128, SLOT], BF16)
    XTs = nc.alloc_sbuf_tensor("XTs", [128, KD * NTOK], BF16)     # 16K
    WhS = nc.alloc_sbuf_tensor("WhS", [128, KH * H], BF16)        # 16K
    WxS = nc.alloc_sbuf_tensor("WxS", [128, KD * H], BF16)        # 4K
    WoS = nc.alloc_sbuf_tensor("WoS", [128, KH * O], BF16)        # 2K
    bS = nc.alloc_sbuf_tensor("bS", [128, MH], FP32)              # 32
    boS = nc.alloc_sbuf_tensor("boS", [128, 1], FP32)             # 4
    IdS = nc.alloc_sbuf_tensor("IdS", [128, 128], BF16)           # 256
    YTs = nc.alloc_sbuf_tensor("YTs", [128, 2 * NT], FP32)        # 4K
    # k-major h copy, 2-block ring: HK[:, k*1024 + (blk%2)*512 + (t%64)*8 + b]
    HK = nc.alloc_sbuf_tensor("HK", [128, KH * 2 * NT], BF16)     # 16K

    # ---- PSUM ----
    P1 = [nc.alloc_psum_tensor(f"P1_{i}", [128, NT], FP32) for i in range(2)]
    PA = [nc.alloc_psum_tensor(f"PA_{i}", [128, SLOT // 2], FP32) for i in range(2)]
    PB = [nc.alloc_psum_tensor(f"PB_{i}", [128, SLOT // 2], FP32) for i in range(2)]

    # ---- semaphores ----
    WS = nc.alloc_semaphore("ws_v2")    # init DMAs + HS memset
    P1S = nc.alloc_semaphore("p1s_v2")  # phase1 PSUM ready     +1/tile
    A1S = nc.alloc_semaphore("a1s_v2")  # phase1 act done       +1/tile
    ZA = nc.alloc_semaphore("za_v2")    # step group-A PSUM ready  +1/step
    ZB = nc.alloc_semaphore("zb_v2")    # step group-B PSUM ready  +1/step
    TA = nc.alloc_semaphore("ta_v2")    # tanh A done           +1/step
    TB = nc.alloc_semaphore("tb_v2")    # tanh B done           +1/step
    P3S = nc.alloc_semaphore("p3s_v2")  # y PSUM ready          +1/block
    YAS = nc.alloc_semaphore("yas_v2")  # y act done            +1/block
    YDS = nc.alloc_semaphore("yds_v2")  # y marker DMA done     +16/block
    YJS = nc.alloc_semaphore("yjs_v2")  # y data DMA raw ticks (not waited)
    CS = nc.alloc_semaphore("cs_v2")    # DVE HK copy done      +1/step

    for s in (WS, P1S, A1S, ZA, ZB, TA, TB, P3S, YAS, YDS, YJS, CS):
        nc.gpsimd.sem_clear(s)
    nc.all_engine_barrier()

    # ---- init loads ----
    nw = 0
    for k in range(KH):
        nc.sync.dma_start(WhS[:, k * H:(k + 1) * H],
                          wh[k * 128:(k + 1) * 128, :]).then_inc(WS, 16)
        nw += 1
    for k in range(KD):
        nc.sync.dma_start(WxS[:, k * H:(k + 1) * H],
                          wx[k * 128:(k + 1) * 128, :]).then_inc(WS, 16)
        nw += 1
        nc.sync.dma_start(XTs[:, k * NTOK:(k + 1) * NTOK],
                          xT[k * 128:(k + 1) * 128, :]).then_inc(WS, 16)
        nw += 1
    for k in range(KH):
        nc.sync.dma_start(WoS[:, k * O:(k + 1) * O],
                          wo[k * 128:(k + 1) * 128, :]).then_inc(WS, 16)
        nw += 1
    nc.sync.dma_start(IdS[:], idn[:, :]).then_inc(WS, 16)
    with nc.allow_non_contiguous_dma(reason="tiny bias vectors"):
        nc.sync.dma_start(bS[:], bv[:].rearrange("(m p) -> p m", p=128)).then_inc(WS, 16)
        nc.sync.dma_start(boS[:], bov[:].rearrange("(p one) -> p one", one=1)).then_inc(WS, 16)
    nw += 3
    nc.vector.memset(H0[:], 0.0).then_inc(WS, 16)
    nc.vector.memset(H1[:], 0.0).then_inc(WS, 16)
    nw += 2
    WTARGET = 16 * nw

    nc.tensor.wait_ge(WS, WTARGET)
    nc.scalar.wait_ge(WS, WTARGET)

    XP3 = XP[:].rearrange("p (s f) -> p s f", f=SLOT)   # [128, T, 64]

    # ---- phase 1: xproj^T = Wx^T @ x^T + b ----
    # 64 (nt, m) pairs; PSUM ping-pong; Act writes bf16 XP with bias.
    a1_pe = MonotonicSemaphore(nc.tensor, A1S)
    pair = 0
    for nt in range(NTOK // NT):
        for m in range(MH):
            pp = pair % 2
            if pair >= 2:
                a1_pe.wait_inc(1)      # act of pair-2 done -> PSUM free
            for k in range(KD):
                mm = nc.tensor.matmul(
                    P1[pp][:],
                    WxS[:, k * H + 128 * m: k * H + 128 * (m + 1)],
                    XTs[:, k * NTOK + nt * NT: k * NTOK + (nt + 1) * NT],
                    start=(k == 0),
                    stop=(k == KD - 1),
                )
                if k == KD - 1:
                    mm.then_inc(P1S, 1)
            pair += 1
    # Act side of phase 1
    p1_act = MonotonicSemaphore(nc.scalar, P1S)
    pair = 0
    for nt in range(NTOK // NT):
        for m in range(MH):
            pp = pair % 2
            p1_act.wait_inc(1)
            nc.scalar.activation(
                XP3[:, nt * 64:(nt + 1) * 64, 8 * m: 8 * (m + 1)],
                P1[pp][:],
                mybir.ActivationFunctionType.Identity,
                bias=bS[:, m:m + 1],
            ).then_inc(A1S, 1)
            pair += 1

    # PE: remaining phase-1 acts must finish before recurrence PSUM overlap
    # is irrelevant (different PSUM tensors) - but step MMs read XP: gate once.
    a1_pe.wait_inc(2)                 # waits A1S >= 64 (all XP written)

    # ---- recurrence ----
    # step t: PA[j] cols 8m+b (m 0..3) = group A, PB[j] = group B (m 4..7).
    # Gates: A:k0-3 tiles wait TA>=t; A:k4-7, B tiles wait TB>=t.
    # tanh A fires after ZA (mid-stream), tanh B after ZB (stream end).
    rta = nc.tensor.alloc_register("rta")
    nc.tensor.reg_mov(rta, 0)
    za_act = MonotonicSemaphore(nc.scalar, ZA)
    cs_act = MonotonicSemaphore(nc.scalar, CS)
    zb_act = MonotonicSemaphore(nc.scalar, ZB)
    yd_act = MonotonicSemaphore(nc.scalar, YDS)
    p3_act = MonotonicSemaphore(nc.scalar, P3S)
    ya_pe = MonotonicSemaphore(nc.tensor, YAS)
    cs_pe = MonotonicSemaphore(nc.tensor, CS)
    tb_dve = MonotonicSemaphore(nc.vector, TB)
    p3_dve = MonotonicSemaphore(nc.vector, P3S)
    HKr = HK[:].rearrange("p (k s) -> p k s", k=KH)   # [128, 8, 1024]

    def step(blk, iv, rk, j):
        # iv: column units, 8 steps/body; j = 0..7; parity p = j%2
        p = j % 2
        hprev = H0 if p == 0 else H1
        hcur = H1 if p == 0 else H0
        xo = iv + (blk * 64 + j) * SLOT
        # identity MMs open both PSUM groups with xproj (no h dependency)
        nc.tensor.matmul(PA[p][:], IdS[:], XP[:, ds(xo, 32)],
                         start=True, stop=False, skip_group_check=True)
        nc.tensor.matmul(PB[p][:], IdS[:], XP[:, ds(xo + 32, 32)],
                         start=True, stop=False, skip_group_check=True)
        nc.tensor.wait_ge(TA, rta)
        for k in range(KH):
            if k == 4:
                nc.tensor.wait_ge(TB, rta)
            for m in range(4):
                mm = nc.tensor.matmul(
                    PA[p][:, 8 * m: 8 * (m + 1)],
                    WhS[:, k * H + 128 * m: k * H + 128 * (m + 1)],
                    hprev[:, k * 8:(k + 1) * 8],
                    start=False, stop=(k == KH - 1),
                    skip_group_check=True,
                )
                if k == KH - 1 and m == 3:
                    mm.then_inc(ZA, 1)
        for k in range(KH):
            for m in range(4, 8):
                mm = nc.tensor.matmul(
                    PB[p][:, 8 * (m - 4): 8 * (m - 3)],
                    WhS[:, k * H + 128 * m: k * H + 128 * (m + 1)],
                    hprev[:, k * 8:(k + 1) * 8],
                    start=False, stop=(k == KH - 1),
                    skip_group_check=True,
                )
                if k == KH - 1 and m == 7:
                    mm.then_inc(ZB, 1)
        nc.tensor.reg_add(rta, rta, 1)
        # Act: tanh halves -> hcur (static dests); gate on DVE copy of t-1
        za_act.wait_inc(1)
        cs_act.wait()
        cs_act.inc_expected(1)
        nc.scalar.activation(hcur[:, 0:32], PA[p][:],
                             mybir.ActivationFunctionType.Tanh).then_inc(TA, 1)
        zb_act.wait_inc(1)
        nc.scalar.activation(hcur[:, 32:SLOT], PB[p][:],
                             mybir.ActivationFunctionType.Tanh).then_inc(TB, 1)
        # DVE: scatter h_t into the k-major ring for the y matmuls
        tb_dve.wait_inc(1)
        nc.vector.tensor_copy(
            HKr[:, :, ds(rk + j * 8, 8)],
            hcur[:].rearrange("p (kk b) -> p kk b", b=BL),
        ).then_inc(CS, 1)

    rk_reg = nc.vector.alloc_register("rk_hk")
    for blk in range(NBLK):
        nc.vector.reg_mov(rk_reg, (blk % 2) * NT)
        with nc.Fori(0, 64 * SLOT, 8 * SLOT) as iv:
            rk = nc.vector.snap(rk_reg, min_val=0, max_val=2 * NT - 64)
            for j in range(8):
                step(blk, iv, rk, j)
            nc.vector.reg_add(rk_reg, rk_reg, 64)

        # ---- inline y for this block: steps blk*64 .. blk*64+63 ----
        # rhs chunk k: HS slots blk*64+1 .. blk*64+64, cols 8k..8k+8
        pp = blk % 2
        cs_pe.inc_expected(64)
        cs_pe.wait()                   # all 64 HK copies of this block done
        if blk >= 2:
            ya_pe.wait_inc(1)          # y-act of blk-2 done -> P1[pp] free
        for k in range(KH):
            mm = nc.tensor.matmul(
                P1[pp][:],
                WoS[:, k * O:(k + 1) * O],
                HK[:, k * 2 * NT + pp * NT: k * 2 * NT + pp * NT + NT],
                start=(k == 0), stop=(k == KH - 1),
            )
            if k == KH - 1:
                mm.then_inc(P3S, 1)
        # Act: bias add -> YTs ping-pong; SP: DMA out
        p3_act.wait_inc(1)
        if blk >= 2:
            # wait for ALL markers through blk-1 (engine completion order
            # across blocks isn't guaranteed; blk-1's marker <- act blk-1
            # which precedes this act, so never circular)
            yd_act.wait_inc(32 if blk == 2 else 16)
        nc.scalar.activation(YTs[:, pp * NT:(pp + 1) * NT], P1[pp][:],
                             mybir.ActivationFunctionType.Identity,
                             bias=boS[:]).then_inc(YAS, 1)
        nc.sync.wait_ge(YAS, blk + 1)
        nc.sync.dma_start(yT[:, blk * NT:(blk + 1) * NT],
                          YTs[:, pp * NT:(pp + 1) * NT]).then_inc(YJS, 16)
        # same-queue marker: lands strictly after the block's y data
        nc.sync.dma_start(ymark[:, :], YTs[:, pp * NT:pp * NT + 1]) \
            .then_inc(YDS, 16)

    # ---- drain: final y DMAs complete before kernel end ----
    nc.gpsimd.wait_ge(YDS, 16 * NBLK)

    nc.compile()
    return nc


def _get_nc():
    global _cached_nc
    if _cached_nc is None:
        _cached_nc = _build()
    return _cached_nc


def kernel(inputs, Wx, Wh, b, Wo, bo):
    global LAST_RESULTS
    x = np.asarray(inputs, dtype=np.float32)        # [B, T, D]
    nc = _get_nc()

    xT_full = np.ascontiguousarray(x.transpose(2, 1, 0)).astype(bfnp)  # [D, T, B]
    whb = np.asarray(Wh, np.float32).astype(bfnp)
    wxb = np.asarray(Wx, np.float32).astype(bfnp)
    wob = np.asarray(Wo, np.float32).astype(bfnp)
    bf = np.ascontiguousarray(np.asarray(b, np.float32))
    bof = np.ascontiguousarray(np.asarray(bo, np.float32))

    in_maps = []
    for c in range(NCORES):
        xs = np.ascontiguousarray(xT_full[:, :, c * BL:(c + 1) * BL]).reshape(D, NTOK)
        in_maps.append({
            "xT": xs, "wh": whb, "wx": wxb, "wo": wob, "bv": bf, "bov": bof,
            "idn": np.eye(128, dtype=np.float32).astype(bfnp),
        })

    res = run_bass_kernel_spmd(nc, in_maps, list(range(NCORES)))
    LAST_RESULTS = res

    y = np.empty((B, T, O), np.float32)
    for c in range(NCORES):
        ytc = res.results[c]["yT"]                   # [O, T*BL], col = t*BL + b
        y[c * BL:(c + 1) * BL] = ytc.reshape(O, T, BL).transpose(2, 1, 0)
    return y
